# revision 27
# baseline (speedup 1.0000x reference)
"""DAG-SCM Trainium2 kernel.

Computes the reference nn_DAGSCM model: a 128-node topological scan
(x_i = relu(w.x_parents + b) + sigma_i * z_i) over n_samples, with the
per-node noise scale sigma_i calibrated from a tiny pilot pass
(0.1 * IQR, computed on host - it is a [128, 256] problem).

Strategy (memory-bound target):
  - Data-parallel over 8 NeuronCores on the sample axis.
  - Per core, samples live as [128 partitions x F free] tiles; each DAG
    node is one free-dim slice. DAG structure and per-node scalars
    (w0, w1, b, sigma) are baked into the traced Bass program as
    immediates at runtime.
  - Only ancestors of the chosen output nodes are computed (~101 of 128
    nodes for this DAG).
  - Noise rows are pre-scaled by sigma on host, stored fp16 and packed
    partition-major in DRAM so every DMA descriptor is a large
    contiguous run (full 360GB/s).  Output tile and DMA are fp16 (host
    upcasts to f32); halves the dominant output traffic vs f32.
  - Per non-root 2-parent node (b==0 here; stt fallback kept for b!=0),
    decomposed so every op has a DVE fast-mode (2x/4x) form or an Act
    alternative -- scalar_tensor_tensor gets no DVE fast mode, so it is
    avoided:
        q = (w_small/w_large) * v_psmall   (tensor_scalar, 4x)
        t = q + v_plarge                   (tensor_tensor, 2x)
        m = max(w_large * t, 0)            (ts mult+max 4x / Act Relu)
        v = m + sigma*z                    (tensor_tensor, 2x)
    The deeper parent is routed through the unscaled t operand so the
    DAG critical path skips q.  GPSIMD/Pool is NOT used: measured ~5-10x
    slower per op on hardware than the cost model claims (enabling it
    took 36us -> 165us per body).
  - Ops are assigned to DVE/Act by an EFT list scheduler over the op
    graph (cost-model per-op costs, cross-engine semaphore latency,
    z-DMA group readiness), and instructions are emitted in scheduled
    start order as a hint to the Tile scheduler.
  - Chosen nodes with children keep a packed vals tile (so child reads
    stay packed/fast-mode) plus one off-path copy into the interleaved
    output tile [p, f*64 + j]; terminal chosen write their column
    directly.  The final fp16 output DMA is contiguous per partition.
  - Double-buffered output/root pools and z-group rotation let
    back-to-back invocations pipeline: body k+1's z DMAs and early
    compute overlap body k's tail output DMA.
"""

import numpy as np

N_CORES = 8
P = 128  # SBUF partitions
CAL_FRAC = 0.1


def _host_pilot_sigma(W_eff, b, parents, is_root, root_pilot):
    """Noiseless pilot scan + per-node sigma = CAL_FRAC * IQR (host, f32)."""
    n_nodes = len(parents)
    n = root_pilot.shape[1]
    vals = np.zeros((n_nodes, n), np.float32)
    for i in range(n_nodes):
        if is_root[i]:
            v = root_pilot[i].astype(np.float32)
        else:
            h = np.zeros(n, np.float32)
            for p, w in parents[i]:
                h = h + np.float32(w) * vals[p]
            v = np.maximum(h + np.float32(b[i]), np.float32(0.0))
        v = np.where(np.isfinite(v), v, np.float32(0.0))
        vals[i] = v
    q75 = np.quantile(vals.astype(np.float64), 0.75, axis=1)
    q25 = np.quantile(vals.astype(np.float64), 0.25, axis=1)
    sigma = CAL_FRAC * np.maximum(q75 - q25, 1e-6)
    return sigma.astype(np.float32)


def _dag_structure(W, b, par_idx, par_mask, is_root, chosen):
    n_nodes = W.shape[0]
    W_eff = (np.asarray(W, np.float32) * np.asarray(par_mask, np.float32))
    parents = []
    for i in range(n_nodes):
        ps = [
            (int(par_idx[i, j]), float(W_eff[i, j]))
            for j in range(par_idx.shape[1])
            if par_mask[i, j] > 0
        ]
        parents.append(ps)
    # needed = chosen + all ancestors
    needed = set(int(c) for c in chosen)
    for i in range(n_nodes - 1, -1, -1):
        if i in needed and not is_root[i]:
            for p, _ in parents[i]:
                needed.add(p)
    return W_eff, parents, needed


def _build_program(NLOC, parents, is_root, chosen, needed, b, sigma, n_nodes,
                   repeats=1, zdt8=False, n_zgroups=12, out_split=4,
                   crit_slack=-1, vals_bufs=1, z_bufs=1,
                   out_bufs=2, ablate=None, ratio_cap=64.0,
                   cost_over=None, no_act=False, no_pool=True,
                   act_cost=None, dve_scale=1.0):
    """Trace the per-core Bass/Tile program. Returns (nc, z_rows, root_rows)."""
    from concourse import bacc
    import concourse.mybir as mybir
    from concourse.tile import TileContext

    F = NLOC // P
    assert NLOC % P == 0

    f32 = mybir.dt.float32
    f16 = mybir.dt.float16
    zdt = mybir.dt.float8e4 if zdt8 else f16
    AF = mybir.ActivationFunctionType
    OP = mybir.AluOpType

    # columns of the output each node must write (normally 0 or 1)
    node_cols = {}
    for j, c in enumerate(int(c) for c in chosen):
        node_cols.setdefault(c, []).append(j)
    n_out = len(chosen)

    topo = [i for i in range(n_nodes) if i in needed]  # index order == topo
    z_rows = [i for i in topo if not is_root[i]]
    z_row_of = {node: r for r, node in enumerate(z_rows)}
    root_rows = [i for i in topo if is_root[i]]
    root_row_of = {node: r for r, node in enumerate(root_rows)}
    # children count within the needed subgraph
    n_child = {i: 0 for i in topo}
    for i in topo:
        if not is_root[i]:
            for p, _ in parents[i]:
                n_child[p] += 1
    # nodes needing a dedicated SBUF vals tile: every non-root node with
    # children (chosen ones also copy into their output column so child
    # reads stay packed); terminal chosen write their column directly.
    vals_nodes = [i for i in topo if not is_root[i]
                  and (i not in node_cols or n_child[i] > 0)]
    n_z = len(z_rows)
    n_root = len(root_rows)

    # depth/height over the needed subgraph -> critical-path set
    depth = {}
    for i in topo:
        if is_root[i]:
            depth[i] = 0
        else:
            depth[i] = 1 + max(depth[p] for p, _ in parents[i])
    height = {i: 0 for i in topo}
    for i in reversed(topo):
        if not is_root[i]:
            for p, _ in parents[i]:
                height[p] = max(height[p], height[i] + 1)
    maxd = max(depth[i] + height[i] for i in topo)
    critical = {i for i in topo if depth[i] + height[i] >= maxd - crit_slack}

    # per-op engine costs (ns, cost-model units for a [128,256] f16 op)
    COST = {
        ("ts", "DVE"): 127.0, ("ts", "ACT"): 398.0, ("ts", "POOL"): 451.0,
        ("tt", "DVE"): 194.0, ("tt", "POOL"): 603.0,
        ("tt8", "DVE"): 327.0, ("tt8", "POOL"): 603.0,
        ("stt", "DVE"): 327.0,
        ("copy", "DVE"): 127.0, ("copy", "ACT"): 398.0,
        ("copy", "POOL"): 451.0,
        ("relu", "ACT"): 398.0,
        "pen": 200.0,  # penalty for a strided operand (kills fast modes)
    }
    if cost_over:
        COST.update(cost_over)
    if act_cost is not None:
        for k in list(COST):
            if isinstance(k, tuple) and k[1] == "ACT":
                COST[k] = float(act_cost)
    if dve_scale != 1.0:
        for k in list(COST):
            if isinstance(k, tuple) and k[1] == "DVE":
                COST[k] = COST[k] * dve_scale
    if no_act:
        COST = {k: v for k, v in COST.items()
                if not (isinstance(k, tuple) and k[1] == "ACT")}
    if no_pool:
        COST = {k: v for k, v in COST.items()
                if not (isinstance(k, tuple) and k[1] == "POOL")}

    # LP-style makespan target: Act absorbs ts-kind ops, Pool tt-kind ops,
    # each up to T*; offloading below the DVE-value/alt-cost ratio frontier
    # wastes capacity (greedy-by-finish-time does, badly).
    n2 = sum(1 for i in topo if not is_root[i] and len(parents[i]) == 2)
    n1 = sum(1 for i in topo if not is_root[i] and len(parents[i]) == 1)
    ncopy = sum(len(node_cols.get(i, [])) for i in topo
                if n_child[i] > 0 or is_root[i])
    vcost = COST[("tt8" if zdt8 else "tt", "DVE")]
    dve_total = (COST[("ts", "DVE")] * (2 * n2 + n1 + ncopy)
                 + COST[("tt", "DVE")] * n2 + vcost * (n2 + n1))
    r_act = (COST[("ts", "DVE")] / COST[("ts", "ACT")]
             if ("ts", "ACT") in COST else 0.0)
    r_pool = (vcost / COST[("tt", "POOL")]
              if ("tt", "POOL") in COST else 0.0)
    t_star = dve_total / (1.0 + r_act + r_pool)
    # offload affinity: which alternates may take each op kind
    AFFINITY = {"ts": ("ACT", "POOL"), "copy": ("ACT", "POOL"),
                "tt": ("POOL",), "tt8": ("POOL",), "stt": ()}

    nc = bacc.Bacc(None, target_bir_lowering=False)
    z_in = nc.dram_tensor("zin", [P, max(n_z, 1) * F], zdt,
                          kind="ExternalInput")
    root_in = nc.dram_tensor("rootin", [P, max(n_root, 1) * F], f16,
                             kind="ExternalInput")
    out_d = nc.dram_tensor("out", [NLOC, n_out], f16, kind="ExternalOutput")

    # z DMA group boundaries (rows)
    gsz = max(1, (n_z + n_zgroups - 1) // n_zgroups)
    zbounds = [(r0, min(r0 + gsz, n_z)) for r0 in range(0, n_z, gsz)]

    with TileContext(nc) as tc:
        with tc.tile_pool(name="vals", bufs=vals_bufs) as vpool, \
             tc.tile_pool(name="zpool", bufs=z_bufs) as zpool, \
             tc.tile_pool(name="tmp", bufs=16) as tpool, \
             tc.tile_pool(name="rootp", bufs=2) as rpool, \
             tc.tile_pool(name="outp", bufs=out_bufs) as opool:

            def trace_body(rep):
                out_t = opool.tile([P, n_out * F], f16, tag="out",
                                   name=f"out{rep}")
                out_cols = out_t[:].rearrange("p (f j) -> p j f", j=n_out)

                # root rows: one fp16 DMA, vals served as slices of it
                rt = rpool.tile([P, n_root * F], f16, tag="root",
                                name=f"rt{rep}")
                if ablate != "compute_only":
                    nc.sync.dma_start(out=rt[:], in_=root_in[:, :])

                # z rows: partition-major packed in DRAM; a few big DMAs
                zgs = []
                for g, (r0, r1) in enumerate(zbounds):
                    zg = zpool.tile([P, (r1 - r0) * F], zdt, tag=f"zg{g}",
                                    name=f"z{rep}_{g}")
                    if ablate != "compute_only":
                        nc.sync.dma_start(out=zg[:],
                                          in_=z_in[:, r0 * F:r1 * F])
                    zgs.append((r0, r1, zg))

                vtile = {}
                for i in vals_nodes:
                    vtile[i] = vpool.tile([P, F], f16, tag=f"v{i}",
                                          name=f"vt{rep}_{i}")

                def z_ap(node):
                    r = z_row_of[node]
                    for r0, r1, zg in zgs:
                        if r0 <= r < r1:
                            return zg[:, (r - r0) * F:(r - r0 + 1) * F]
                    raise AssertionError

                def col_ap(j):
                    return out_cols[:, j]

                def src_ap(node):
                    """Packed value AP of a node (parents always have one)."""
                    if is_root[node]:
                        rr = root_row_of[node]
                        return rt[:, rr * F:(rr + 1) * F]
                    return vtile[node][:]

                def tmp(nm, i):
                    return tpool.tile([P, F], f16, tag=nm,
                                      name=f"{nm}{rep}_{i}")[:]

                # ---- engine-explicit emitters -------------------------
                def do_ts(e, dst, in0, s1, s2, op0, op1, relu_scale=None):
                    if e == "ACT":
                        if relu_scale is not None:
                            nc.scalar.activation(dst, in0, AF.Relu,
                                                 bias=0.0,
                                                 scale=float(relu_scale))
                        else:
                            nc.scalar.activation(dst, in0, AF.Copy,
                                                 bias=float(s2 or 0.0),
                                                 scale=float(s1))
                        return
                    eng = nc.gpsimd if e == "POOL" else nc.vector
                    eng.tensor_scalar(
                        out=dst, in0=in0, scalar1=float(s1),
                        scalar2=None if s2 is None else float(s2),
                        op0=op0, **({} if s2 is None else {"op1": op1}))

                def do_tt(e, dst, in0, in1):
                    eng = nc.gpsimd if e == "POOL" else nc.vector
                    eng.tensor_tensor(out=dst, in0=in0, in1=in1, op=OP.add)

                def do_copy(e, dst, in0):
                    if e == "ACT":
                        nc.scalar.activation(dst, in0, AF.Copy,
                                             bias=0.0, scale=1.0)
                    elif e == "POOL":
                        nc.gpsimd.tensor_copy(out=dst, in_=in0)
                    else:
                        nc.vector.tensor_copy(out=dst, in_=in0)

                def do_stt(e, dst, in0, scalar, in1, op0, op1):
                    nc.vector.scalar_tensor_tensor(
                        out=dst, in0=in0, scalar=float(scalar), in1=in1,
                        op0=op0, op1=op1)

                def node_plan(i):
                    """Return (kind, data) describing how to compute i."""
                    ps = parents[i]
                    bi = float(b[i])
                    if len(ps) == 0:
                        return ("zonly", max(bi, 0.0))
                    if len(ps) == 1 and bi == 0.0:
                        return ("one", ps[0])
                    if len(ps) == 2 and bi == 0.0:
                        (pa, wa), (pb, wb) = ps[0], ps[1]
                        da = depth[pa] + height.get(pa, 0)
                        db = depth[pb] + height.get(pb, 0)
                        if da != db:
                            (pl, wl), (psm, wsm) = (
                                ((pa, wa), (pb, wb)) if da > db
                                else ((pb, wb), (pa, wa)))
                            if wl == 0.0 or abs(wsm / wl) > ratio_cap:
                                (pl, wl), (psm, wsm) = (
                                    ((pa, wa), (pb, wb))
                                    if abs(wa) >= abs(wb)
                                    else ((pb, wb), (pa, wa)))
                        else:
                            (pl, wl), (psm, wsm) = (
                                ((pa, wa), (pb, wb)) if abs(wa) >= abs(wb)
                                else ((pb, wb), (pa, wa)))
                        if wl == 0.0:
                            return ("zonly", 0.0)
                        return ("two", (pl, wl, psm, wsm / wl))
                    return ("fallback", (ps, bi))

                # ---- phase 1: symbolic op graph -----------------------
                # ops[k]: {menu, deps (earlier op ids), zg, root, emit}
                ops = []

                def add_op(kind, deps, emitfn, crit=False, strided=False,
                           zg=None, root_dep=False):
                    menu = {}
                    for e in ("DVE", "ACT", "POOL"):
                        c = COST.get((kind, e))
                        if c is None or (crit and e != "DVE"):
                            continue
                        if e == "DVE" and strided:
                            c += COST["pen"]
                        menu[e] = c
                    ops.append(dict(menu=menu, deps=deps, emit=emitfn,
                                    zg=zg, root=root_dep))
                    return len(ops) - 1

                def zg_of(node):
                    r = z_row_of[node]
                    for g, (r0, r1) in enumerate(zbounds):
                        if r0 <= r < r1:
                            return g
                    raise AssertionError

                val_op = {}   # node -> op id producing its value

                def pdep(p):
                    return ([] if is_root[p] else [val_op[p]],
                            is_root[p])

                if ablate != "dma_only":
                    for r in root_rows:
                        for j in node_cols.get(r, []):
                            add_op("copy", [], (
                                lambda e, _j=j, _r=r:
                                do_copy(e, col_ap(_j), src_ap(_r))),
                                strided=True, root_dep=True)
                    for i in topo:
                        if is_root[i]:
                            continue
                        kind, data = node_plan(i)
                        crit = i in critical
                        zs = z_ap(i)
                        zg = zg_of(i)
                        if i in vtile:
                            dst, dst_str = vtile[i][:], False
                        else:
                            dst, dst_str = col_ap(node_cols[i][0]), True
                        if kind == "zonly":
                            cval = data
                            vid = add_op("ts", [], (
                                lambda e, _d=dst, _z=zs, _c=cval:
                                do_ts(e, _d, _z, 1.0, _c, OP.mult,
                                      OP.add)), crit, dst_str, zg=zg)
                        elif kind == "one":
                            p0, w0 = data
                            d0, rt0 = pdep(p0)
                            m_t = tmp("m", i)
                            mid = add_op("ts", d0, (
                                lambda e, _m=m_t, _p=src_ap(p0), _w=w0:
                                do_ts(e, _m, _p, _w, 0.0, OP.mult, OP.max,
                                      relu_scale=_w)), crit,
                                root_dep=rt0)
                            vid = add_op("tt8" if zdt8 else "tt", [mid], (
                                lambda e, _d=dst, _m=m_t, _z=zs:
                                do_tt(e, _d, _m, _z)), crit, dst_str,
                                zg=zg)
                        elif kind == "two":
                            pl, wl, psm, ratio = data
                            dl, rtl = pdep(pl)
                            dsm, rtsm = pdep(psm)
                            if ratio != 0.0:
                                q_t = tmp("q", i)
                                qid = add_op("ts", dsm, (
                                    lambda e, _q=q_t, _p=src_ap(psm),
                                    _r=ratio:
                                    do_ts(e, _q, _p, _r, None, OP.mult,
                                          None)), crit, root_dep=rtsm)
                                t_t = tmp("t", i)
                                tid = add_op("tt", dl + [qid], (
                                    lambda e, _t=t_t, _q=q_t,
                                    _p=src_ap(pl):
                                    do_tt(e, _t, _q, _p)), crit,
                                    root_dep=rtl)
                                t_src, tdeps = t_t, [tid]
                            else:
                                t_src, tdeps = src_ap(pl), dl
                            m_t = tmp("m", i)
                            mid = add_op("ts", tdeps, (
                                lambda e, _m=m_t, _t=t_src, _w=wl:
                                do_ts(e, _m, _t, _w, 0.0, OP.mult, OP.max,
                                      relu_scale=_w)), crit,
                                root_dep=(rtl if ratio == 0.0 else False))
                            vid = add_op("tt8" if zdt8 else "tt", [mid], (
                                lambda e, _d=dst, _m=m_t, _z=zs:
                                do_tt(e, _d, _m, _z)), crit, dst_str,
                                zg=zg)
                        else:
                            ps, bi = data
                            (p0, w0) = ps[0]
                            (p1, w1) = ps[-1]
                            d1, rt1 = pdep(p1)
                            u_t = tmp("q", i)
                            uid = add_op("ts", d1, (
                                lambda e, _u=u_t, _p=src_ap(p1), _w=w1,
                                _b=bi:
                                do_ts(e, _u, _p, _w, _b, OP.mult,
                                      OP.add)), crit, root_dep=rt1)
                            sdeps = [uid]
                            s_src = u_t
                            if len(ps) == 2:
                                d0, rt0 = pdep(p0)
                                s_t = tmp("t", i)
                                sid = add_op("stt", d0 + [uid], (
                                    lambda e, _s=s_t, _p=src_ap(p0),
                                    _w=w0, _u=u_t:
                                    do_stt(e, _s, _p, _w, _u, OP.mult,
                                           OP.add)), crit, root_dep=rt0)
                                sdeps, s_src = [sid], s_t
                            vid = add_op("stt", sdeps, (
                                lambda e, _d=dst, _s=s_src, _z=zs:
                                do_stt(e, _d, _s, 0.0, _z, OP.max,
                                       OP.add)), crit, dst_str, zg=zg)
                        val_op[i] = vid
                        if i in vtile:
                            for j in node_cols.get(i, []):
                                add_op("copy", [vid], (
                                    lambda e, _j=j, _i=i:
                                    do_copy(e, col_ap(_j), vtile[_i][:])),
                                    strided=True)
                        elif i in node_cols:
                            for j in node_cols[i][1:]:
                                add_op("copy", [vid], (
                                    lambda e, _j=j, _j0=node_cols[i][0]:
                                    do_copy(e, col_ap(_j), col_ap(_j0))),
                                    strided=True)

                # ---- phase 2: EFT list schedule -----------------------
                n_ops = len(ops)
                SEM_LAT = 130.0
                DMA_SEM = 900.0
                # DMA readiness: SP issues root then z groups serially;
                # DMA engines serve them in order at aggregate bandwidth.
                bw = 22.5 * 16.0
                cursor = 0.0
                served = 1400.0
                root_bytes = P * n_root * F * 2
                served = max(served, cursor + 1400.0) + root_bytes / bw
                root_ready = served + DMA_SEM
                z_ready = []
                for g, (r0, r1) in enumerate(zbounds):
                    cursor += 600.0
                    zbytes = P * (r1 - r0) * F * (1 if zdt8 else 2)
                    served = max(served, cursor + 1400.0) + zbytes / bw
                    z_ready.append(served + DMA_SEM)
                if ablate == "compute_only":
                    root_ready = 0.0
                    z_ready = [0.0] * len(z_ready)

                children = [[] for _ in range(n_ops)]
                indeg = [0] * n_ops
                for oi, op in enumerate(ops):
                    for d in op["deps"]:
                        children[d].append(oi)
                        indeg[oi] += 1
                mincost = [min(op["menu"].values()) for op in ops]
                hgt = [0.0] * n_ops
                for oi in range(n_ops - 1, -1, -1):
                    hgt[oi] = mincost[oi] + max(
                        (hgt[c] + SEM_LAT for c in children[oi]),
                        default=0.0)
                import heapq
                heap = []
                for oi in range(n_ops):
                    if indeg[oi] == 0:
                        heapq.heappush(heap, (-hgt[oi], oi))
                avail = {"DVE": 0.0, "ACT": 0.0, "POOL": 0.0}
                start = [0.0] * n_ops
                finish = [0.0] * n_ops
                eng_of = [None] * n_ops
                while heap:
                    _, oi = heapq.heappop(heap)
                    op = ops[oi]
                    base = 0.0
                    if op["zg"] is not None:
                        base = z_ready[op["zg"]]
                    if op["root"]:
                        base = max(base, root_ready)
                    best = None
                    for e, c in op["menu"].items():
                        rdy = base
                        for d in op["deps"]:
                            lat = 0.0 if eng_of[d] == e else SEM_LAT
                            rdy = max(rdy, finish[d] + lat)
                        f = max(avail[e], rdy) + c
                        if best is None or f < best[0]:
                            best = (f, e)
                    f, e = best
                    start[oi], finish[oi], eng_of[oi] = f - op["menu"][e], f, e
                    avail[e] = f
                    for c in children[oi]:
                        indeg[c] -= 1
                        if indeg[c] == 0:
                            heapq.heappush(heap, (-hgt[c], c))
                nc._sched_pred = max(finish) if n_ops else 0.0
                sums = {"DVE": 0.0, "ACT": 0.0, "POOL": 0.0}
                cnts = {"DVE": 0, "ACT": 0, "POOL": 0}
                for oi, op in enumerate(ops):
                    sums[eng_of[oi]] += op["menu"][eng_of[oi]]
                    cnts[eng_of[oi]] += 1
                nc._sched_stats = (dict(avail), sums, cnts,
                                   root_ready, list(z_ready))

                # ---- phase 3: emit in scheduled start order -----------
                for oi in sorted(range(n_ops), key=lambda k: start[k]):
                    ops[oi]["emit"](eng_of[oi])

                # output DMA: all 128 partitions per transfer, split along
                # the free dim across several dma_starts
                out_ap = out_d[:, :].rearrange("(p f) j -> p (f j)", p=P)
                FS = (F + out_split - 1) // out_split
                for f0 in range(0, F, FS):
                    f1 = min(f0 + FS, F)
                    nc.sync.dma_start(
                        out=out_ap[:, f0 * n_out:f1 * n_out],
                        in_=out_t[:, f0 * n_out:f1 * n_out])

            for rep in range(repeats):
                trace_body(rep)

    nc.finalize()
    return nc, z_rows, root_rows


_CACHE = {}
_LAST_NC = None
_LAST_IN_MAPS = None


def _get_program(key, *args, **kwargs):
    if key not in _CACHE:
        _CACHE[key] = _build_program(*args, **kwargs)
    return _CACHE[key]


def run(n_samples, W, b, root_pilot, root_main, z_noise, par_mask, par_idx,
        is_root, chosen, trace=False, n_cores=N_CORES, repeats=1,
        **build_kwargs):
    import ml_dtypes

    W = np.asarray(W, np.float32)
    b = np.asarray(b, np.float32)
    root_pilot = np.asarray(root_pilot, np.float32)
    root_main = np.asarray(root_main, np.float32)
    z_noise = np.asarray(z_noise, np.float32)
    par_mask = np.asarray(par_mask, np.float32)
    par_idx = np.asarray(par_idx, np.int32)
    is_root = np.asarray(is_root, bool)
    chosen = np.asarray(chosen, np.int32)

    n_nodes = W.shape[0]
    NS = root_main.shape[1]
    assert NS % (n_cores * P) == 0
    NLOC = NS // n_cores
    F = NLOC // P

    W_eff, parents, needed = _dag_structure(W, b, par_idx, par_mask, is_root,
                                            chosen)
    sigma = _host_pilot_sigma(W_eff, b, parents, is_root, root_pilot)

    zdt8 = build_kwargs.get("zdt8", False)
    key = (NLOC, n_nodes, tuple(chosen.tolist()), par_idx.tobytes(),
           par_mask.tobytes(), W_eff.tobytes(), b.tobytes(), sigma.tobytes(),
           is_root.tobytes(), repeats, tuple(sorted(build_kwargs.items())))
    nc, z_rows, root_rows = _get_program(
        key, NLOC, parents, is_root, chosen, needed, b, sigma, n_nodes,
        repeats=repeats, **build_kwargs)

    zdt = ml_dtypes.float8_e4m3 if zdt8 else np.float16
    n_z, n_root = len(z_rows), len(root_rows)
    if n_z:
        zsel = (z_noise[z_rows] * sigma[z_rows][:, None]).astype(zdt)
    else:
        zsel = np.zeros((1, NS), zdt)
    rsel = (root_main[root_rows].astype(np.float16) if n_root
            else np.zeros((1, NS), np.float16))

    def pack(a, c):
        # [rows, NLOC] slice for core c -> [P, rows*F] partition-major
        s = a[:, c * NLOC:(c + 1) * NLOC]
        r = s.shape[0]
        return np.ascontiguousarray(
            s.reshape(r, P, F).transpose(1, 0, 2).reshape(P, r * F))

    in_maps = [{"zin": pack(zsel, c), "rootin": pack(rsel, c)}
               for c in range(n_cores)]

    from concourse.bass_utils import run_bass_kernel_spmd
    global _LAST_NC, _LAST_IN_MAPS
    _LAST_NC, _LAST_IN_MAPS = nc, in_maps
    res = run_bass_kernel_spmd(nc, in_maps, core_ids=list(range(n_cores)),
                               trace=trace)
    out = np.concatenate([np.asarray(r["out"]) for r in res.results], axis=0)
    return out.astype(np.float32), res


def kernel(**inputs):
    out, _ = run(**inputs)
    return out
